# revision 11
# baseline (speedup 1.0000x reference)
"""Multi-head attention (Swin-style relative position bias) on 8 Trainium2 cores.

Sharding: pure data parallel — batch B=8, one batch element per NeuronCore.
Weights (qkv_w, proj_w, proj_b) and the relative-position bias are replicated.

Per-core pipeline (all matmuls in fp32r = fp22 mantissa, fp32 accumulate):
  1. x (1024,768) -> PE-transpose -> xT_big (C on partitions)
  2. qkT = qkv_w[:, :1536].T @ xT   (q/k in head-transposed layout)
     v    = x @ qkv_w[:, 1536:]    (token-major, interleaved with a ones
                                    column per head -> PV matmul also emits
                                    softmax row sums)
  3. per head h, per row tile t (128 rows):
       S = qT.T @ kT  (PSUM) ; S += 8*bias  (DVE, Toeplitz-expanded tile)
       P = exp(0.125*S)  (ACT, PSUM->SBUF)
       PT = PE-transpose(P)  -> PT_big   (unnormalized)
     per head: outT_unnorm/sums = [v|1].T @ PT ; outT = outT_unnorm * recip(sums)
       (DVE reciprocal + rank-1 PE broadcast + DVE multiply)
  4. outFT = proj_w.T @ outT + proj_b   (bias via ACT per-partition add)
  5. PE-transpose outFT -> natural layout -> DMA out

The (1024,1024) bias matrix per head is block-Toeplitz: every row tile is a
slice of one (128, 1920) tile B8[h] precomputed on host from the 63x63 used
part of bias_table (pre-scaled by 8 so the 1/8 softmax scale can be folded
into the ACT exp).
"""

import os
import sys

sys.path.insert(0, "/opt/trn_rl_repo")

import numpy as np
from contextlib import ExitStack

import concourse.bass as bass
import concourse.bacc as bacc
import concourse.mybir as mybir
import concourse.tile as tile

F32 = mybir.dt.float32
F32R = mybir.dt.float32r

N = 1024          # tokens
C = 768           # channels
NH = 12           # heads
HD = 64           # head dim
HW = 32           # H = W = 32
NT = N // 128     # 8 token tiles
KC = C // 128     # 6 contraction chunks
GB = 60           # B8 tile free blocks of 32


def _seg(ap, seg_size, seg, lo, size):
    """ap[:, seg*seg_size + lo : ... + size]"""
    return ap[:, seg * seg_size + lo: seg * seg_size + lo + size]


def build_nc():
    nc = bacc.Bacc()

    x_d = nc.dram_tensor("x", [N, C], F32, kind="ExternalInput")
    qkvw_d = nc.dram_tensor("qkvw", [C, 3 * C], F32, kind="ExternalInput")
    projw_d = nc.dram_tensor("projw", [C, C], F32, kind="ExternalInput")
    projb_d = nc.dram_tensor("projb", [128, KC], F32, kind="ExternalInput")
    b8_d = nc.dram_tensor("b8", [NH, 128, GB * 32], F32, kind="ExternalInput")
    consts_d = nc.dram_tensor("consts", [128, 224], F32, kind="ExternalInput")
    out_d = nc.dram_tensor("out", [N, C], F32, kind="ExternalOutput")

    r = F32R

    with tile.TileContext(nc) as tc, ExitStack() as ctx:
        const = ctx.enter_context(tc.tile_pool(name="const", bufs=1))
        main = ctx.enter_context(tc.tile_pool(name="main", bufs=1))

        consts = const.tile([128, 224], F32R, tag="consts")
        nc.sync.dma_start(consts[:], consts_d[:].bitcast(F32R))
        ident = consts[:, 0:128]
        ones = consts[:, 128:192]
        projb_sb = const.tile([128, KC], F32, tag="projb")
        nc.sync.dma_start(projb_sb[:], projb_d[:])
        # persistent big tiles
        qk_big = main.tile([128, 12 * N], F32R, tag="qk")        # 12 segs of (128,1024)
        v_big = main.tile([128, NT * NH * (HD + 1)], F32R, tag="v")  # 8 segs of (128,780)
        outT_big = main.tile([128, KC * N], F32R, tag="outT")    # 6 segs (c-chunks)
        VSEG = NH * (HD + 1)  # 780

        # ---------------- phase 1+2: xT, qkT, v ----------------
        with (
            tc.tile_pool(name="ph2", bufs=1) as ph2,
            tc.tile_pool(name="xload", bufs=2) as xload,
            tc.tile_pool(name="ps_x", bufs=2, space="PSUM") as ps_x,
            tc.tile_pool(name="ps_qk", bufs=2, space="PSUM") as ps_qk,
            tc.tile_pool(name="ps_v", bufs=2, space="PSUM") as ps_v,
        ):
            qkvw_sb = ph2.tile([128, KC * 3 * C], F32R, tag="qkvw")
            nc.sync.dma_start(
                qkvw_sb[:].rearrange("p (kc e) -> p kc e", kc=KC),
                qkvw_d[:].bitcast(F32R).rearrange("(kc p) e -> p kc e", p=128),
            )
            xT_big = ph2.tile([128, KC * N], F32R, tag="xT")  # 6 segs (c-chunks)

            nc.vector.tensor_copy(
                v_big[:].rearrange("p (s h e) -> p s h e", s=NT, h=NH)[:, :, :, HD:HD + 1],
                consts[:, 128:224].rearrange("p (s h e) -> p s h e", s=NT, h=NH),
            )

            for t in range(NT):
                xt = xload.tile([128, C], F32R, tag="xt")
                nc.sync.dma_start(xt[:], x_d[t * 128:(t + 1) * 128, :].bitcast(F32R))
                for c0, cn in ((0, 4), (4, 2)):
                    ps = ps_x.tile([128, cn * 128], F32R, tag="psx")
                    for ci in range(cn):
                        nc.tensor.transpose(
                            ps[:, ci * 128:(ci + 1) * 128],
                            xt[:, (c0 + ci) * 128:(c0 + ci + 1) * 128],
                            ident,
                        )
                    dst = xT_big[:].rearrange("p (c i) -> p c i", c=KC)[
                        :, c0:c0 + cn, t * 128:(t + 1) * 128]
                    nc.vector.tensor_copy(dst, ps[:])

            # qkT: 12 M-tiles x 2 N-chunks, accumulate over 6 K-chunks
            for m in range(12):
                for n in range(2):
                    ps = ps_qk.tile([128, 512], F32, tag="psqk")
                    for kc in range(KC):
                        nc.tensor.matmul(
                            ps[:],
                            _seg(qkvw_sb[:], 3 * C, kc, m * 128, 128),
                            _seg(xT_big[:], N, kc, n * 512, 512),
                            start=(kc == 0), stop=(kc == KC - 1),
                        )
                    nc.vector.tensor_copy(
                        _seg(qk_big[:], N, m, n * 512, 512), ps[:])

            # v: token-major, into v_big head blocks (cols 65h..65h+64)
            for t in range(NT):
                for n in range(2):  # v-dim chunks of 512 and 256
                    nsz = 512 if n == 0 else 256
                    ps = ps_v.tile([128, nsz], F32, tag="psv")
                    for kc in range(KC):
                        nc.tensor.matmul(
                            ps[:],
                            _seg(xT_big[:], N, kc, t * 128, 128),
                            _seg(qkvw_sb[:], 3 * C, kc, 2 * C + n * 512, nsz),
                            start=(kc == 0), stop=(kc == KC - 1),
                        )
                    dst = v_big[:].rearrange("p (s h e) -> p s h e", s=NT, h=NH)[
                        :, t, n * 8:n * 8 + nsz // HD, 0:HD]
                    nc.vector.tensor_copy(dst, ps[:])

        # ---------------- phase 3: attention per head ----------------
        with (
            tc.tile_pool(name="b8", bufs=2) as b8p,
            tc.tile_pool(name="pt", bufs=1) as ptp,
            tc.tile_pool(name="pp", bufs=3) as pp,
            tc.tile_pool(name="rr", bufs=2) as rrp,
            tc.tile_pool(name="ps_s", bufs=2, space="PSUM") as ps_s,
            tc.tile_pool(name="ps_t", bufs=2, space="PSUM") as ps_t,
            tc.tile_pool(name="ps_o", bufs=2, space="PSUM") as ps_o,
        ):
            for h in range(NH):
                qoff = (h % 2) * 64
                qseg = h // 2
                kseg = 6 + h // 2

                b8h = b8p.tile([128, GB * 32], F32, tag="b8")
                nc.sync.dma_start(b8h[:], b8_d[h])

                PT_big = ptp.tile([128, NT * N], F32R, tag="PT")  # 8 segs (j-chunks)

                for t in range(NT):
                    psS = ps_s.tile([128, N], F32, tag="psS")
                    for n in range(2):
                        nc.tensor.matmul(
                            psS[:, n * 512:(n + 1) * 512],
                            _seg(qk_big[:], N, qseg, t * 128, 128)[qoff:qoff + 64, :],
                            _seg(qk_big[:], N, kseg, n * 512, 512)[qoff:qoff + 64, :],
                            start=True, stop=True,
                        )
                    # bias add: B8 slice at free offset 32*(28-4t) = 896-128t
                    nc.vector.tensor_add(
                        psS[:], psS[:], b8h[:, 896 - 128 * t: 1920 - 128 * t])
                    pt_tile = pp.tile([128, N], F32R, tag="P")
                    nc.scalar.activation(
                        pt_tile[:], psS[:], mybir.ActivationFunctionType.Exp,
                        scale=0.125)
                    for c0, cn in ((0, 4), (4, 4)):
                        ps = ps_t.tile([128, cn * 128], F32R, tag="psT")
                        for ci in range(cn):
                            nc.tensor.transpose(
                                ps[:, ci * 128:(ci + 1) * 128],
                                pt_tile[:, (c0 + ci) * 128:(c0 + ci + 1) * 128],
                                ident,
                            )
                        dst = PT_big[:].rearrange("p (c i) -> p c i", c=NT)[
                            :, c0:c0 + cn, t * 128:(t + 1) * 128]
                        nc.vector.tensor_copy(dst, ps[:])

                # PV with fused row sums: lhsT = [v_h | 1] (128, 65)
                for ih in range(2):
                    psO = ps_o.tile([65, 512], F32, tag="psO")
                    for c in range(NT):
                        nc.tensor.matmul(
                            psO[:],
                            v_big[:, c * VSEG + h * (HD + 1): c * VSEG + (h + 1) * (HD + 1)],
                            _seg(PT_big[:], N, c, ih * 512, 512),
                            start=(c == 0), stop=(c == NT - 1),
                        )
                    rr = rrp.tile([65, 512], F32R, tag="rr")
                    with nc.allow_low_precision(reason="fp32r matmul feed"):
                        nc.vector.reciprocal(rr[64:65, :], psO[64:65, :])
                    psB = ps_o.tile([64, 512], F32, tag="psO")
                    nc.tensor.matmul(
                        psB[:], ones[64:65, :], rr[64:65, :],
                        start=True, stop=True,
                    )
                    bc = rrp.tile([64, 512], F32, tag="bc")
                    nc.vector.tensor_copy(bc[:], psB[:])
                    nc.vector.scalar_tensor_tensor(
                        _seg(outT_big[:], N, h // 2, ih * 512, 512)[qoff:qoff + 64, :],
                        psO[0:64, :], 1.0, bc[:],
                        op0=mybir.AluOpType.bypass, op1=mybir.AluOpType.mult,
                    )

        # ---------------- phase 4+5: projection, bias, final transpose ----------------
        with (
            tc.tile_pool(name="ph45", bufs=1) as ph45,
            tc.tile_pool(name="onat", bufs=2) as onat,
            tc.tile_pool(name="ps_f", bufs=2, space="PSUM") as ps_f,
            tc.tile_pool(name="ps_n", bufs=2, space="PSUM") as ps_n,
        ):
            projw_sb = ph45.tile([128, KC * C], F32R, tag="projw")
            nc.sync.dma_start(
                projw_sb[:].rearrange("p (kc e) -> p kc e", kc=KC),
                projw_d[:].bitcast(F32R).rearrange("(kc p) e -> p kc e", p=128),
            )
            outFT_big = ph45.tile([128, KC * N], F32R, tag="outFT")  # 6 segs (e-chunks)
            for em in range(KC):
                for n in range(2):
                    ps = ps_f.tile([128, 512], F32, tag="psF")
                    for kc in range(KC):
                        nc.tensor.matmul(
                            ps[:],
                            _seg(projw_sb[:], C, kc, em * 128, 128),
                            _seg(outT_big[:], N, kc, n * 512, 512),
                            start=(kc == 0), stop=(kc == KC - 1),
                        )
                    nc.scalar.activation(
                        _seg(outFT_big[:], N, em, n * 512, 512), ps[:],
                        mybir.ActivationFunctionType.Identity,
                        bias=projb_sb[:, em:em + 1], scale=1.0)

            for t in range(NT):
                on = onat.tile([128, C], F32, tag="onat")
                for e0, en in ((0, 4), (4, 2)):
                    ps = ps_n.tile([128, en * 128], F32R, tag="psN")
                    for ei in range(en):
                        nc.tensor.transpose(
                            ps[:, ei * 128:(ei + 1) * 128],
                            _seg(outFT_big[:], N, e0 + ei, t * 128, 128),
                            ident,
                        )
                    nc.vector.tensor_copy(on[:, e0 * 128:(e0 + en) * 128], ps[:])
                nc.sync.dma_start(out_d[t * 128:(t + 1) * 128, :], on[:])

    nc.compile()
    return nc


def make_b8(bias_table: np.ndarray) -> np.ndarray:
    """(65025, 12) bias table -> (12, 128, 1920) pre-scaled Toeplitz-expanded
    row-tile source. B8[h][p, 32*g + jw] = 8 * T_h[p//32 - g + 59, p%32 - jw + 31],
    where T_h = bias_table[:3969, h].reshape(63, 63). Row tile t of the full
    (1024,1024) bias matrix for head h is B8[h][:, 896-128t : 1920-128t].
    """
    t8 = np.ascontiguousarray(
        (8.0 * np.asarray(bias_table[:3969], np.float32)).T)  # (12, 3969)
    out = np.empty((NH, 128, GB * 32), np.float32)
    s = t8.strides[-1]
    for h in range(NH):
        base = t8[h]
        view = np.lib.stride_tricks.as_strided(
            base[3748:], shape=(4, 32, GB, 32),
            strides=(63 * s, s, -63 * s, -s))
        out[h] = view.reshape(128, GB * 32)
    return out


_NC_CACHE = None
LAST_EXEC_NS = None


def _install_trace_shim():
    """Provide antenv.axon_hooks (missing in this image) so
    run_bass_kernel_spmd(trace=True) can NTFF-profile via the axon .so."""
    import sys as _sys
    if "antenv.axon_hooks" in _sys.modules:
        return
    try:
        import types
        import antenv
        from trn_agent_boot.trn_boot import _ntff_profile_via_ctypes
        hook = _ntff_profile_via_ctypes("/opt/axon/libaxon_pjrt.so")
        mod = types.ModuleType("antenv.axon_hooks")
        mod._hook = hook
        mod.get_axon_ntff_profile_hook = lambda: mod._hook
        mod.set_axon_ntff_profile_hook = lambda h: setattr(mod, "_hook", h)
        _sys.modules["antenv.axon_hooks"] = mod
        antenv.axon_hooks = mod
    except Exception as e:  # tracing is best-effort
        print(f"trace shim unavailable: {e}")


def kernel(x, qkv_w, proj_w, proj_b, bias_table):
    global _NC_CACHE, LAST_EXEC_NS
    from concourse.bass_utils import run_bass_kernel_spmd

    x = np.asarray(x, np.float32)
    qkv_w = np.ascontiguousarray(np.asarray(qkv_w, np.float32))
    proj_w = np.ascontiguousarray(np.asarray(proj_w, np.float32))
    proj_b = np.asarray(proj_b, np.float32)
    bias_table = np.asarray(bias_table, np.float32)

    B = x.shape[0]
    assert x.shape == (B, N, C) and B == 8

    if _NC_CACHE is None:
        _NC_CACHE = build_nc()
    nc = _NC_CACHE

    b8 = make_b8(bias_table)
    projb_l = np.ascontiguousarray(proj_b.reshape(KC, 128).T)  # (128, 6)

    consts = np.concatenate(
        [np.eye(128, dtype=np.float32), np.ones((128, 96), np.float32)], axis=1)
    shared = {"qkvw": qkv_w, "projw": proj_w, "projb": projb_l, "b8": b8,
              "consts": consts}
    in_maps = [dict(shared, x=np.ascontiguousarray(x[b])) for b in range(B)]

    trace = bool(int(os.environ.get("KERNEL_TRACE", "0")))
    tmpdir = None
    if trace:
        import tempfile
        sys.path.insert(0, "/root/.axon_site")
        _install_trace_shim()
        base = os.environ.get("KERNEL_TRACE_DIR")
        if base:
            os.makedirs(base, exist_ok=True)
            tmpdir = tempfile.mkdtemp(prefix="cap", dir=base)
        else:
            tmpdir = tempfile.mkdtemp(prefix="ktrace")
        print("trace dir:", tmpdir)
    res = run_bass_kernel_spmd(
        nc, in_maps, core_ids=list(range(B)), trace=trace, tmpdir=tmpdir,
    )
    LAST_EXEC_NS = res.exec_time_ns
    return np.stack([res.results[b]["out"] for b in range(B)], axis=0)
